# revision 1
# baseline (speedup 1.0000x reference)
"""Trainium2 Bass kernel for nn_GATrEncoder (B=8, N=1024, H=128 channels, 16-comp multivectors).

Sharding: pure data-parallel over the batch dim B=8 -> one batch element per
NeuronCore (8 cores), no collectives needed.

The GATr block collapses dramatically under blade-component analysis of
Cl(3,0,1) with the fixed embeddings used by this model:

  * The final output extracts only the scalar component of
    equi_linear(x, w_out); equi_linear is component-diagonal, so only the
    scalar component of the last residual stream matters.
  * The residual stream before the MLP has zero scalar component, and the
    outer-product ("join") branch has zero scalar component, so its
    scalar-gated-gelu gate gelu(0)=0 kills the whole join branch.
  * Only the non-degenerate (e0-free) blade components {e12,e13,e23,e123}
    ever influence norms, attention logits, or the geometric product's
    scalar output.  pts and tx feed only degenerate components -> both are
    dead inputs.  Only view (normalized direction d) matters.
  * Attention collapses to 8 heads over 4-dim features
    K[s] = [f1*dz, f1*dy, f1*dx, f1] with per-head query scales
    qs_H = scale * [P_H, P_H, P_H, Q_H] (data-independent scalars derived
    from the weights), and V = [1 | K] (the 1 gives the softmax denominator).
  * Everything after attention is a chain of 9-dim linear maps per token,
    a quadratic form for the second layer norm, a 4-term diagonal geometric
    product for the scalar output, gelu gating, and a final 128x64 linear.

Large matmuls run as float32r (e8m11, full-rate fp32 on the PE with fp32
PSUM accumulation); small preprocessing matmuls as plain fp32.  Measured
against the fp32 jax reference on trn2 hardware: relative error 1.3e-3
(CoreSim, which does not model fp32r rounding: 9e-7).  Modeled per-core
execution time ~121 us; the 8 cores run the 8 batch elements in parallel.
"""

import numpy as np

import concourse.bass as bass
import concourse.tile as tile
import concourse.mybir as mybir
from concourse import bacc
from concourse.bass_utils import run_bass_kernel_spmd

F32 = mybir.dt.float32
F32R = mybir.dt.float32r
AF = mybir.ActivationFunctionType

B = 8
N = 1024
NCH = 8          # token chunks of 128
NH = 8           # attention heads
SCALE = float(1.0 / np.sqrt(128.0))

WNAMES = ["w_in", "w_q", "w_k", "w_v", "w_attn_out", "w_mlp_in",
          "w_gp_l", "w_gp_r", "w_mlp_out", "w_out"]


def _host_consts():
    """Data-independent constant tensors fed to every core (two packed blobs)."""
    # blob_f (f32 consts): ident | head_ind | qsp | qsq | ones128 | e0col | shift8
    blob_f = np.zeros((128, 522), np.float32)
    blob_f[:, 0:128] = np.eye(128, dtype=np.float32)
    for h in range(NH):
        blob_f[16 * h:16 * h + 16, 128 + h] = 1.0          # head_ind
        blob_f[h, 136 + 4 * h:136 + 4 * h + 3] = SCALE     # qsp
        blob_f[h, 168 + 4 * h + 3] = SCALE                 # qsq
    blob_f[0, 200:328] = 1.0                               # ones128 (1,128)
    blob_f[0, 328] = 1.0                                   # e0col (1,9) col0
    for i in range(8):
        blob_f[i, 337 + 1 + i] = 1.0                       # shift8 (8,9): [i, i+1]
    # blob_r (f32r-consumed consts): psel (44,36) | dsel44 (44,44) | ones64 | ones128c
    blob_r = np.zeros((128, 36 + 44 + 64 + 1), np.float32)
    # S layout (44 rows): head h rows 5h..5h+4 = [den, nz, ny, nx, ng];
    # rows 40..43 = [dz, dy, dx, 1].
    for k in range(4):
        blob_r[40 + k, 9 * k] = 1.0
        for h in range(NH):
            blob_r[5 * h + 1 + k, 9 * k + 1 + h] = 1.0     # psel
    # dsel44: divisor-broadcast in S-row space: rows 5h+1..4 get den_h; others 1
    for h in range(NH):
        for j in range(5):
            blob_r[5 * h, 36 + 5 * h + j] = 1.0
    for r2 in range(40, 44):
        blob_r[43, 36 + r2] = 1.0
    blob_r[0, 80:144] = 1.0                                # ones64 (1,64)
    blob_r[:, 144] = 1.0                                   # ones128c (128,1)
    # pselT (9, 4*44) in blob_f cols 346:522: pselT[j, 44k+src] = psel[src, 9k+j]
    for k in range(4):
        blob_f[0, 346 + 44 * k + 40 + k] = 1.0
        for h in range(NH):
            blob_f[1 + h, 346 + 44 * k + 5 * h + 1 + k] = 1.0
    return {"blob_f": blob_f, "blob_r": blob_r}


def _mmr(nc, out, lhsT, rhs, **kw):
    """matmul in float32r (full-rate fp32 at N>=256) with fp32 PSUM accumulation."""
    nc.tensor.matmul(out, lhsT.bitcast(F32R), rhs.bitcast(F32R), **kw)


def _mm(nc, out, lhsT, rhs, **kw):
    """plain fp32 matmul (small-N preprocessing ops)."""
    nc.tensor.matmul(out, lhsT, rhs, **kw)


def _emit(tc):
    nc = tc.nc
    t = {}
    t["view"] = nc.declare_dram_parameter("view", [N, 3], F32, isOutput=False)
    t["w_in"] = nc.declare_dram_parameter("w_in", [5, 128, 2], F32, isOutput=False)
    for w in ["w_q", "w_k", "w_v", "w_attn_out", "w_mlp_in", "w_mlp_out", "w_out"]:
        t[w] = nc.declare_dram_parameter(w, [5, 128, 128], F32, isOutput=False)
    for w in ["w_gp_l", "w_gp_r"]:
        t[w] = nc.declare_dram_parameter(w, [5, 64, 128], F32, isOutput=False)
    for cname, arr in _host_consts().items():
        t[cname] = nc.declare_dram_parameter(cname, list(arr.shape), F32, isOutput=False)
    out_d = nc.declare_dram_parameter("out", [N, 128], F32, isOutput=True)

    HN = 512

    with tc.tile_pool(name="sg", bufs=1) as sg, \
         tc.tile_pool(name="wraw", bufs=4) as wraw:

        # ------- critical-path DMAs first (sync queue) -------
        vt = sg.tile([128, NCH, 3], F32, tag="vt")
        nc.sync.dma_start(out=vt, in_=t["view"][:, :].rearrange("(c p) j -> p c j", p=128))
        bf = sg.tile([128, 522], F32, tag="bf")
        nc.sync.dma_start(out=bf, in_=t["blob_f"][:, :])
        a_sb = sg.tile([128, 1], F32, tag="a_sb")
        nc.sync.dma_start(out=a_sb, in_=t["w_in"][2, :, 0:1])
        b_sb = sg.tile([128, 1], F32, tag="b_sb")
        nc.sync.dma_start(out=b_sb, in_=t["w_in"][3, :, 1:2])
        early_raw = {}
        for nm, wn, g in [("wq2", "w_q", 2), ("wk2", "w_k", 2),
                          ("wq3", "w_q", 3), ("wk3", "w_k", 3)]:
            raw = wraw.tile([128, 128], F32, tag="wload", name="raw_" + nm)
            nc.sync.dma_start(out=raw, in_=t[wn][g, :, :])
            early_raw[nm] = raw
        br = sg.tile([128, 145], F32, tag="br")
        nc.sync.dma_start(out=br.bitcast(F32R), in_=t["blob_r"][:, :].bitcast(F32R))

        ident = bf[:, 0:128]
        head_ind = bf[:, 128:136]
        qsp = bf[0:8, 136:168]
        qsq = bf[0:8, 168:200]
        ones128 = bf[0:1, 200:328]
        e0col = bf[0:1, 328:337]
        shift8 = bf[0:8, 337:346]
        psel = br[0:44, 0:36]
        dsel44 = br[0:44, 36:80]
        ones64 = br[0:1, 80:144]
        ones128c = br[:, 144:145]

        WT = {}
        S = sg.tile([44, N], F32, tag="S")
        with tc.tile_pool(name="pp", bufs=4, space="PSUM") as pp:
            # ---- ma/mb scalar broadcasts ----
            def mean_sq_bcast(vec, nm, bias):
                ps = pp.tile([1, 1], F32, tag="pp")
                _mm(nc, ps, vec, vec)
                sb1 = sg.tile([1, 1], F32, tag="ms_" + nm, name="ms_" + nm)
                nc.scalar.mul(out=sb1, in_=ps, mul=1.0 / 128.0)
                bc = pp.tile([128, 1], F32, tag="pp")
                _mm(nc, bc, ones128, sb1)
                outt = sg.tile([128, 1], F32, tag="msb_" + nm, name="msb_" + nm)
                nc.scalar.activation(out=outt, in_=bc, func=AF.Copy, bias=bias)
                return outt

            ma_t = mean_sq_bcast(a_sb, "ma", 0.0)
            mbe_t = mean_sq_bcast(b_sb, "mb", 1e-5)

            # ---- stage A ----
            Dall = sg.tile([128, NCH, 4], F32, tag="Dall")
            Kall = sg.tile([128, NCH, 5], F32, tag="Kall")
            sqv = sg.tile([128, NCH, 3], F32, tag="sqv")
            nc.scalar.activation(out=sqv, in_=vt, func=AF.Square)
            n2 = sg.tile([128, NCH], F32, tag="n2")
            nc.vector.tensor_reduce(out=n2, in_=sqv, axis=mybir.AxisListType.X,
                                    op=mybir.AluOpType.add)
            nrm = sg.tile([128, NCH], F32, tag="nrm")
            nc.scalar.activation(out=nrm, in_=n2, func=AF.Sqrt)
            den = sg.tile([128, NCH], F32, tag="den")
            nc.vector.tensor_scalar_add(out=den, in0=nrm, scalar1=1e-9)
            rcp = sg.tile([128, NCH], F32, tag="rcp")
            nc.vector.reciprocal(out=rcp, in_=den)
            for j in range(3):
                nc.gpsimd.tensor_mul(out=Dall[:, :, j], in0=vt[:, :, j], in1=rcp)
            nc.gpsimd.memset(Dall[:, :, 3], 1.0)
            ts1 = sg.tile([128, NCH], F32, tag="ts1")
            nc.vector.tensor_mul(out=ts1, in0=n2, in1=rcp)
            ts2 = sg.tile([128, NCH], F32, tag="ts2")
            nc.vector.tensor_mul(out=ts2, in0=ts1, in1=rcp)
            ts3 = sg.tile([128, NCH], F32, tag="ts3")
            nc.vector.tensor_scalar(out=ts3, in0=ts2, scalar1=ma_t, scalar2=mbe_t,
                                    op0=mybir.AluOpType.mult, op1=mybir.AluOpType.add)
            sq2 = sg.tile([128, NCH], F32, tag="sq2")
            nc.scalar.activation(out=sq2, in_=ts3, func=AF.Sqrt)
            f1 = sg.tile([128, NCH], F32, tag="f1")
            nc.vector.reciprocal(out=f1, in_=sq2)
            for j in range(3):
                nc.vector.tensor_mul(out=Kall[:, :, 1 + j].bitcast(F32R),
                                     in0=Dall[:, :, j], in1=f1)
            nc.vector.tensor_copy(out=Kall[:, :, 4].bitcast(F32R), in_=f1)
            nc.vector.tensor_scalar(out=Kall[:, :, 0].bitcast(F32R), in0=f1,
                                    scalar1=0.0, scalar2=1.0,
                                    op0=mybir.AluOpType.mult, op1=mybir.AluOpType.add)

            KT = sg.tile([4, N], F32, tag="KT")
            for c in range(NCH):
                ps = pp.tile([4, 128], F32, tag="ppt")
                nc.tensor.transpose(ps, Kall[:, c, 1:5], ident)
                nc.scalar.copy(out=KT[:, 128 * c:128 * c + 128].bitcast(F32R), in_=ps)

            # ---- attention head scalars ----
            def wtrans_from(raw, nm, rows=128):
                ps = pp.tile([128, rows], F32, tag="pp")
                nc.tensor.transpose(ps, raw, ident[0:rows, 0:rows])
                wt = sg.tile([128, rows], F32, tag="wt_" + nm, name="wt_" + nm)
                nc.vector.tensor_copy(out=wt, in_=ps)
                return wt

            for nm in ["wq2", "wk2", "wq3", "wk3"]:
                WT[nm] = wtrans_from(early_raw[nm], nm)

            def mat_vec(wt, vec, nm):
                ps = pp.tile([128, 1], F32, tag="pp")
                _mm(nc, ps, wt, vec)
                sb = sg.tile([128, 1], F32, tag="mv_" + nm, name="mv_" + nm)
                nc.vector.tensor_copy(out=sb, in_=ps)
                return sb

            Aq = mat_vec(WT["wq2"], a_sb, "aq")
            Ak = mat_vec(WT["wk2"], a_sb, "ak")
            Bq = mat_vec(WT["wq3"], b_sb, "bq")
            Bk = mat_vec(WT["wk3"], b_sb, "bk")
            z_sb = sg.tile([128, 1], F32, tag="z_sb")
            nc.vector.tensor_mul(out=z_sb, in0=Aq, in1=Ak)
            zb_sb = sg.tile([128, 1], F32, tag="zb_sb")
            nc.vector.tensor_mul(out=zb_sb, in0=Bq, in1=Bk)
            P8ps = pp.tile([8, 1], F32, tag="pp")
            _mm(nc, P8ps, head_ind, z_sb)
            P8 = sg.tile([8, 1], F32, tag="P8")
            nc.vector.tensor_copy(out=P8, in_=P8ps)
            Q8ps = pp.tile([8, 1], F32, tag="pp")
            _mm(nc, Q8ps, head_ind, zb_sb)
            Q8 = sg.tile([8, 1], F32, tag="Q8")
            nc.vector.tensor_copy(out=Q8, in_=Q8ps)
            qs = []
            for h in range(NH):
                ps = pp.tile([4, 1], F32, tag="pp")
                _mm(nc, ps, qsp[:, 4 * h:4 * h + 4], P8, start=True, stop=False)
                _mm(nc, ps, qsq[:, 4 * h:4 * h + 4], Q8, start=False, stop=True)
                sb = sg.tile([4, 1], F32, tag=f"qs{h}", name=f"qs{h}")
                nc.vector.tensor_copy(out=sb, in_=ps)
                qs.append(sb)

        # ---------------- attention + overlapped late preprocessing ----
        with tc.tile_pool(name="xp", bufs=2) as xp, \
             tc.tile_pool(name="lpp", bufs=2, space="PSUM") as lpp, \
             tc.tile_pool(name="avpp", bufs=1, space="PSUM") as avpp, \
             tc.tile_pool(name="pp2", bufs=2, space="PSUM") as pp2:
            for h in range(NH):
                qv = xp.tile([4, N], F32, tag="qv", bufs=3)
                nc.vector.tensor_scalar_mul(out=qv.bitcast(F32R), in0=KT, scalar1=qs[h])
                expl = xp.tile([128, NCH, N], F32, tag="expl")
                for s in range(NCH):
                    lp = lpp.tile([128, N], F32, tag="lp")
                    for h2 in range(2):
                        _mmr(nc, lp[:, 512 * h2:512 * h2 + 512],
                             KT[:, 128 * s:128 * s + 128],
                             qv[:, 512 * h2:512 * h2 + 512])
                    nc.scalar.activation(out=expl[:, s, :].bitcast(F32R), in_=lp,
                                         func=AF.Exp)
                avp = avpp.tile([5, N], F32, tag="avp")
                for s in range(NCH):
                    for h2 in range(2):
                        _mmr(nc, avp[:, 512 * h2:512 * h2 + 512],
                             Kall[:, s, :],
                             expl[:, s, 512 * h2:512 * h2 + 512],
                             start=(s == 0), stop=(s == NCH - 1))
                avsb = xp.tile([5, N], F32, tag="avsb", bufs=2)
                for h2 in range(2):
                    sl = slice(512 * h2, 512 * h2 + 512)
                    nc.vector.tensor_copy(out=avsb[:, sl].bitcast(F32R),
                                          in_=avp[:, sl])
                    nc.sync.dma_start(out=S[5 * h:5 * h + 5, sl].bitcast(F32R),
                                      in_=avsb[:, sl].bitcast(F32R))

            # ---- late preprocessing (scheduler fills attention gaps) ----
            def wtrans_late(nm, wn, g, rows):
                raw = wraw.tile([rows, 128], F32, tag="wload", name="raw_" + nm)
                nc.gpsimd.dma_start(out=raw, in_=t[wn][g, :, :])
                ps = pp2.tile([128, rows], F32, tag="pp2")
                nc.tensor.transpose(ps, raw, ident[0:rows, 0:rows])
                wt = sg.tile([128, rows], F32, tag="wt_" + nm, name="wt_" + nm)
                nc.vector.tensor_copy(out=wt, in_=ps)
                return wt

            for nm, wn, g, rows in [("wv2", "w_v", 2, 128), ("wv3", "w_v", 3, 128),
                                    ("wao2", "w_attn_out", 2, 128),
                                    ("wao3", "w_attn_out", 3, 128),
                                    ("wmi2", "w_mlp_in", 2, 128),
                                    ("wmi3", "w_mlp_in", 3, 128),
                                    ("wgl2", "w_gp_l", 2, 64), ("wgl3", "w_gp_l", 3, 64),
                                    ("wgr2", "w_gp_r", 2, 64), ("wgr3", "w_gp_r", 3, 64),
                                    ("wout0", "w_out", 0, 128)]:
                WT[nm] = wtrans_late(nm, wn, g, rows)
            wmlpout0 = sg.tile([128, 128], F32, tag="wmlpout0")
            nc.gpsimd.dma_start(out=wmlpout0, in_=t["w_mlp_out"][0, :, :])

            def mat_vec2(wt, vec, nm):
                ps = pp2.tile([128, 1], F32, tag="pp2")
                _mm(nc, ps, wt, vec)
                sb = sg.tile([128, 1], F32, tag="mv_" + nm, name="mv_" + nm)
                nc.vector.tensor_copy(out=sb, in_=ps)
                return sb

            Av = mat_vec2(WT["wv2"], a_sb, "av")
            Bv = mat_vec2(WT["wv3"], b_sb, "bv")
            avm = sg.tile([128, 8], F32, tag="avm")
            nc.vector.tensor_scalar_mul(out=avm, in0=head_ind, scalar1=Av)
            bvm = sg.tile([128, 8], F32, tag="bvm")
            nc.vector.tensor_scalar_mul(out=bvm, in0=head_ind, scalar1=Bv)

            # E matrices, transposed form ET (9,128) = [a^T ; C^T]
            aT_ps = pp2.tile([1, 128], F32, tag="pp2")
            nc.tensor.transpose(aT_ps, a_sb, ident)
            aT_sb = sg.tile([1, 128], F32, tag="aT_sb")
            nc.vector.tensor_copy(out=aT_sb, in_=aT_ps)
            bT_ps = pp2.tile([1, 128], F32, tag="pp2")
            nc.tensor.transpose(bT_ps, b_sb, ident)
            bT_sb = sg.tile([1, 128], F32, tag="bT_sb")
            nc.vector.tensor_copy(out=bT_sb, in_=bT_ps)
            CT_ps = pp2.tile([8, 128], F32, tag="pp2")
            _mm(nc, CT_ps, avm, WT["wao2"])
            CT_sb = sg.tile([8, 128], F32, tag="CT_sb")
            nc.vector.tensor_copy(out=CT_sb, in_=CT_ps)
            DT_ps = pp2.tile([8, 128], F32, tag="pp2")
            _mm(nc, DT_ps, bvm, WT["wao3"])
            DTp_sb = sg.tile([8, 128], F32, tag="DTp_sb")
            nc.vector.tensor_copy(out=DTp_sb, in_=DT_ps)

            ET_ps = pp2.tile([9, 128], F32, tag="pp2")
            _mm(nc, ET_ps, e0col, aT_sb, start=True, stop=False)
            _mm(nc, ET_ps, shift8, CT_sb, start=False, stop=True)
            ET = sg.tile([9, 128], F32, tag="ET")
            nc.vector.tensor_copy(out=ET.bitcast(F32R), in_=ET_ps)
            EpT_ps = pp2.tile([9, 128], F32, tag="pp2")
            _mm(nc, EpT_ps, e0col, bT_sb, start=True, stop=False)
            _mm(nc, EpT_ps, shift8, DTp_sb, start=False, stop=True)
            EpT = sg.tile([9, 128], F32, tag="EpT")
            nc.vector.tensor_copy(out=EpT.bitcast(F32R), in_=EpT_ps)

            # E_sb (128,9) still needed for T1 = Wmi @ E
            E_sb = sg.tile([128, 9], F32, tag="E_sb")
            Ep_sb = sg.tile([128, 9], F32, tag="Ep_sb")
            Cps = pp2.tile([128, 8], F32, tag="pp2")
            _mm(nc, Cps, WT["wao2"], avm)
            nc.vector.tensor_copy(out=E_sb[:, 0:1], in_=a_sb)
            nc.vector.tensor_copy(out=E_sb[:, 1:9], in_=Cps)
            Dps = pp2.tile([128, 8], F32, tag="pp2")
            _mm(nc, Dps, WT["wao3"], bvm)
            nc.vector.tensor_copy(out=Ep_sb[:, 0:1], in_=b_sb)
            nc.vector.tensor_copy(out=Ep_sb[:, 1:9], in_=Dps)

            T1ps = pp2.tile([128, 9], F32, tag="pp2")
            _mm(nc, T1ps, WT["wmi2"], E_sb)
            T1 = sg.tile([128, 9], F32, tag="T1")
            nc.vector.tensor_copy(out=T1, in_=T1ps)
            T1pps = pp2.tile([128, 9], F32, tag="pp2")
            _mm(nc, T1pps, WT["wmi3"], Ep_sb)
            T1p = sg.tile([128, 9], F32, tag="T1p")
            nc.vector.tensor_copy(out=T1p, in_=T1pps)

            LR = {}
            for nm, lhsT, rhs in [("L2T", T1, WT["wgl2"]), ("R2T", T1, WT["wgr2"]),
                                  ("L3T", T1p, WT["wgl3"]), ("R3T", T1p, WT["wgr3"])]:
                ps = pp2.tile([9, 64], F32, tag="pp2")
                _mm(nc, ps, lhsT, rhs)
                LR[nm] = sg.tile([9, 64], F32, tag="lr_" + nm, name="lr_" + nm)
                nc.vector.tensor_copy(out=LR[nm].bitcast(F32R), in_=ps)

            FTps = pp2.tile([64, 128], F32, tag="pp2")
            _mm(nc, FTps, wmlpout0[:, 0:64], WT["wout0"])
            FT = sg.tile([64, 128], F32, tag="FT")
            nc.vector.tensor_copy(out=FT.bitcast(F32R), in_=FTps)

            # D^T rows of S (40..43)
            DT = sg.tile([4, N], F32, tag="DT")
            for c in range(NCH):
                ps2 = pp2.tile([4, 128], F32, tag="pp2")
                nc.tensor.transpose(ps2, Dall[:, c, :], ident)
                nc.vector.tensor_copy(out=DT[:, 128 * c:128 * c + 128].bitcast(F32R),
                                      in_=ps2)
            nc.gpsimd.dma_start(out=S[40:44, :].bitcast(F32R), in_=DT.bitcast(F32R))



            # composed selection matrices: stage-C matmuls consume S~ directly
            pselT = bf[0:9, 346:522]
            CS = {}
            for k in range(4):
                pT = pselT[:, 44 * k:44 * k + 44]
                for nm, rhs in [("ES", ET if k < 3 else EpT),
                                ("LS", LR["L2T"] if k < 3 else LR["L3T"]),
                                ("RS", LR["R2T"] if k < 3 else LR["R3T"])]:
                    cols = rhs.shape[-1]
                    ps = pp2.tile([44, cols], F32, tag="pp2")
                    _mm(nc, ps, pT, rhs.bitcast(F32))
                    sb = sg.tile([44, cols], F32, tag=f"{nm}{k}", name=f"{nm}{k}")
                    nc.vector.tensor_copy(out=sb.bitcast(F32R), in_=ps)
                    CS[(nm, k)] = sb

        # ---------------- stage C ----------------
        with tc.tile_pool(name="cp", bufs=4, space="PSUM") as cp, \
             tc.tile_pool(name="q2p", bufs=2, space="PSUM") as q2p, \
             tc.tile_pool(name="tpp", bufs=2, space="PSUM") as tpp, \
             tc.tile_pool(name="csb", bufs=1) as csb:
            HL = [slice(0, HN), slice(HN, N)]
            St = csb.tile([44, N], F32, tag="St")
            Srec = csb.tile([44, N], F32, tag="Srec")
            for h2 in range(2):
                divb = cp.tile([44, HN], F32, tag="cps")
                _mmr(nc, divb, dsel44, S[:, HL[h2]])
                nc.vector.reciprocal(out=Srec[:, HL[h2]], in_=divb)
                nc.vector.tensor_mul(out=St[:, HL[h2]].bitcast(F32R), in0=S[:, HL[h2]],
                                     in1=Srec[:, HL[h2]])

            # f2 path: q2 = sum_k ||E Yk||^2 (per token)
            negf2 = csb.tile([1, N], F32, tag="negf2")
            rf2 = csb.tile([1, N], F32, tag="rf2")
            f2b = csb.tile([64, N], F32, tag="f2b")
            for h2 in range(2):
                q2 = q2p.tile([1, HN], F32, tag="q2ps")
                for k in range(4):
                    xps = cp.tile([128, HN], F32, tag="cps")
                    _mmr(nc, xps, CS[("ES", k)], St[:, HL[h2]])
                    sqx = csb.tile([128, HN], F32, tag="sqx", bufs=3)
                    nc.scalar.activation(out=sqx.bitcast(F32R), in_=xps, func=AF.Square)
                    _mmr(nc, q2, ones128c, sqx, start=(k == 0), stop=(k == 3))
                nc.scalar.activation(out=negf2[:, HL[h2]], in_=q2, func=AF.Copy,
                                     scale=-1.0 / 128.0, bias=-1e-5)
                with nc.allow_low_precision(reason="f32r rounding for PE"):
                    nc.vector.reciprocal(out=rf2[:, HL[h2]].bitcast(F32R),
                                         in_=negf2[:, HL[h2]])
                f2ps = cp.tile([64, HN], F32, tag="cps")
                _mmr(nc, f2ps, ones64, rf2[:, HL[h2]])
                nc.scalar.copy(out=f2b[:, HL[h2]], in_=f2ps)

            # gp scalar accumulation
            acc = csb.tile([64, N], F32, tag="acc")
            for h2 in range(2):
                for k in range(4):
                    lps = cp.tile([64, HN], F32, tag="cps")
                    _mmr(nc, lps, CS[("LS", k)], St[:, HL[h2]])
                    lsb = csb.tile([64, HN], F32, tag="lsb", bufs=2)
                    nc.any.tensor_copy(out=lsb, in_=lps)
                    rps = cp.tile([64, HN], F32, tag="cps")
                    _mmr(nc, rps, CS[("RS", k)], St[:, HL[h2]])
                    if k == 0:
                        nc.vector.tensor_mul(out=acc[:, HL[h2]], in0=lsb, in1=rps)
                    else:
                        prod = csb.tile([64, HN], F32, tag="prod", bufs=2)
                        nc.vector.tensor_mul(out=prod, in0=lsb, in1=rps)
                        nc.gpsimd.tensor_add(out=acc[:, HL[h2]], in0=acc[:, HL[h2]],
                                             in1=prod)

            # gg = x*gelu_tanh(x) = 0.5*x^2*(1 + tanh(c2*x*(1 + c1*x^2)))
            c1 = 0.044715
            c2 = float(np.sqrt(2.0 / np.pi))
            gp0 = csb.tile([64, N], F32, tag="gp0")
            u2 = csb.tile([64, N], F32, tag="u2")
            pv = csb.tile([64, N], F32, tag="pv")
            inner = csb.tile([64, N], F32, tag="inner")
            th = csb.tile([64, N], F32, tag="th")
            qq = csb.tile([64, N], F32, tag="qq")
            gg = csb.tile([64, N], F32, tag="gg")
            osb = csb.tile([128, N], F32, tag="osb")
            osb2 = csb.tile([128, NCH, 128], F32, tag="osb2")
            for q in range(4):
                sl = slice(256 * q, 256 * q + 256)
                nc.vector.tensor_mul(out=gp0[:, sl], in0=acc[:, sl], in1=f2b[:, sl])
                nc.scalar.activation(out=u2[:, sl], in_=gp0[:, sl], func=AF.Square)
                nc.scalar.activation(out=pv[:, sl], in_=u2[:, sl], func=AF.Copy,
                                     scale=c2 * c1, bias=c2)
                nc.vector.tensor_mul(out=inner[:, sl], in0=gp0[:, sl], in1=pv[:, sl])
                nc.scalar.activation(out=th[:, sl], in_=inner[:, sl], func=AF.Tanh)
                nc.scalar.activation(out=qq[:, sl], in_=th[:, sl], func=AF.Copy,
                                     scale=0.5, bias=0.5)
                nc.vector.tensor_mul(out=gg[:, sl].bitcast(F32R), in0=u2[:, sl],
                                     in1=qq[:, sl])
                op = cp.tile([128, 256], F32, tag="cps")
                _mmr(nc, op, FT, gg[:, sl])
                nc.scalar.copy(out=osb[:, sl], in_=op)
                for c in (2 * q, 2 * q + 1):
                    tp = tpp.tile([128, 128], F32, tag="tp")
                    nc.tensor.transpose(tp, osb[:, 128 * c:128 * c + 128], ident)
                    nc.any.tensor_copy(out=osb2[:, c, :], in_=tp)
            nc.sync.dma_start(
                out=out_d[:, :].rearrange("(c p) o -> p c o", p=128), in_=osb2)


def build_nc():
    nc = bacc.Bacc()
    with tile.TileContext(nc) as tc:
        _emit(tc)
    nc.finalize()
    return nc


_BUILT = None


def _get_built():
    global _BUILT
    if _BUILT is None:
        _BUILT = build_nc()
    return _BUILT


def kernel(**inputs):
    nc = _get_built()
    consts = _host_consts()
    base = dict(consts)
    for w in WNAMES:
        base[w] = np.ascontiguousarray(np.asarray(inputs[w], np.float32))
    view = np.asarray(inputs["view"], np.float32)
    in_maps = []
    for c in range(B):
        m = dict(base)
        m["view"] = np.ascontiguousarray(view[c])
        in_maps.append(m)
    res = run_bass_kernel_spmd(nc, in_maps, core_ids=list(range(B)))
    return np.stack([res.results[c]["out"] for c in range(B)], axis=0)

